# revision 14
# baseline (speedup 1.0000x reference)
"""Fused multi-head causal attention block (qkv proj + attention + out proj)
for Trainium2, data-parallel over batch across 8 NeuronCores.

Contract: kernel(**inputs) takes the FULL inputs
    x [8,1024,1024] f32, Wqkv [1024,3072], bqkv [3072], Wproj [1024,1024],
    bproj [1024]
and returns (a, present) exactly like the reference:
    a [8,1024,1024] f32, present [2,8,16,1024,64] f32.

Per-core program (SPMD, one batch element per core):
  stage A: qkv projection.  q^T,k^T produced transposed [hd, s] (head-major
           rows); v produced in natural [s, hd] layout padded with a ones
           column per head (width 65) so the context matmul can also yield
           the softmax denominator.
  attn:    scores computed transposed  w^T[t,s] = (k^T).T @ (q^T)  per head,
           causal blocks skipped, additive -1e4 mask on diagonal blocks,
           exp without max-subtraction (scores are O(1) here).  Context
           accumulated over t-chunks into one PSUM bank per (head, s-chunk):
           even heads write rows 0..64 via lhsT=[v|1] (row 64 = denominator),
           odd heads write rows 64..127 via lhsT=v plus a separate
           denominator group on row 0 of the same bank.  Normalize via a
           K=1 ones-matmul broadcast of 1/denom + DVE multiply, keeping every
           engine lane-aligned; head pairs share [128, S] aT tiles.
  proj:    out[s,m] = sum over 128-row hd chunks  aT[c].T @ Wproj[c]  + bias.

All matmul operands are bitcast to float32r (full PE rate at N=512).
Biases are applied exactly via K=1 rank-1 update matmuls.
"""

import numpy as np

import concourse.bass as bass
import concourse.mybir as mybir
import concourse.tile as tile
from concourse import bacc

B, S, NX, H, D = 8, 1024, 1024, 16, 64
P = 128
E = D + 1  # v columns per head incl. ones column
F32 = mybir.dt.float32
FR = mybir.dt.float32r
AF = mybir.ActivationFunctionType
OP = mybir.AluOpType


def _fr(ap):
    return ap.bitcast(FR)


def build_nc():
    nc = bacc.Bacc("TRN2", target_bir_lowering=False)

    xT_d = nc.dram_tensor("xT", [NX, S], F32, kind="ExternalInput")
    wqkv_d = nc.dram_tensor("Wqkv", [NX, 3 * NX], F32, kind="ExternalInput")
    bqkv_d = nc.dram_tensor("bqkv", [1, 3 * NX], F32, kind="ExternalInput")
    wproj_d = nc.dram_tensor("Wproj", [NX, NX], F32, kind="ExternalInput")
    bproj_d = nc.dram_tensor("bproj", [1, NX], F32, kind="ExternalInput")
    mask_d = nc.dram_tensor("cmask4", [4, P, 512], F32, kind="ExternalInput")
    ones_d = nc.dram_tensor("ones", [1, S], F32, kind="ExternalInput")
    vones_d = nc.dram_tensor("vones", [P, H], F32, kind="ExternalInput")
    outa_d = nc.dram_tensor("out_a", [S, NX], F32, kind="ExternalOutput")
    outk_d = nc.dram_tensor("out_kT", [NX, S], F32, kind="ExternalOutput")
    outv_d = nc.dram_tensor("out_v", [S, NX], F32, kind="ExternalOutput")

    with tile.TileContext(nc) as tc:
        with (
            tc.tile_pool(name="const", bufs=1) as constp,
            tc.tile_pool(name="qk", bufs=1, side="right") as qkp,
            tc.tile_pool(name="vpp", bufs=1, side="right") as vpp,
            tc.tile_pool(name="atp", bufs=1) as atp,
        ):
            # Memset cannot produce fp32r-rounded data, so all-ones constants
            # consumed by matmuls are DMA'd from DRAM inputs instead.
            ones_sb = constp.tile([1, S], F32, name="ones_sb")
            nc.sync.dma_start(out=_fr(ones_sb), in_=_fr(ones_d[:, :]))
            # ones row on partition 64 for the 1/denom broadcast matmul
            # (lhsT must share the rhs rec row's base partition)
            ones64 = constp.tile([P, D], F32, name="ones64")
            nc.sync.dma_start(out=_fr(ones64[D : D + 1, :]), in_=_fr(ones_d[:, 0:D]))
            # full-width additive causal masks, one per diagonal position:
            # cols < q*128 get -1e4 (fully masked), the 128-wide diagonal
            # block gets the triangular mask, cols beyond it get 0
            mask_sb = [
                constp.tile([P, 512], F32, name=f"mask_sb{q}", tag=f"mask_sb{q}")
                for q in range(4)
            ]
            for q in range(4):
                nc.sync.dma_start(out=mask_sb[q], in_=mask_d[q])

            qT = [qkp.tile([P, S], F32, name=f"qT{i}", tag=f"qT{i}") for i in range(8)]
            kT = [qkp.tile([P, S], F32, name=f"kT{i}", tag=f"kT{i}") for i in range(8)]
            vpad = [
                vpp.tile([P, H * E], F32, name=f"vp{i}", tag=f"vp{i}") for i in range(8)
            ]
            # head pair c: head 2c on partitions 0..63, head 2c+1 on 64..127
            aT = [atp.tile([P, S], F32, name=f"aT{c}", tag=f"aT{c}") for c in range(8)]

            # ---------------- stage A: qkv = x @ Wqkv + b ----------------
            with (
                tc.tile_pool(name="xp", bufs=1) as xp,
                tc.tile_pool(name="wgp", bufs=12) as wgp,
                tc.tile_pool(name="psA", bufs=5, space="PSUM") as psA,
            ):
                xT = []
                for n in range(8):
                    t = xp.tile([P, S], F32, name=f"xT{n}", tag=f"xT{n}")
                    nc.sync.dma_start(out=_fr(t), in_=_fr(xT_d[n * P : (n + 1) * P, :]))
                    xT.append(t)
                for t in range(8):
                    v3 = vpad[t].rearrange("p (h e) -> p h e", e=E)
                    nc.sync.dma_start(
                        out=_fr(v3[:, :, D : D + 1]), in_=_fr(vones_d[:, :, None])
                    )

                for mg in range(6):
                    wg = []
                    for n in range(8):
                        w = wgp.tile([P, 512], F32, name=f"wg_{mg}_{n}", tag="wg")
                        nc.sync.dma_start(
                            out=_fr(w),
                            in_=_fr(
                                wqkv_d[n * P : (n + 1) * P, mg * 512 : (mg + 1) * 512]
                            ),
                        )
                        wg.append(w)
                    bqg = wgp.tile([1, 512], F32, name=f"bqg_{mg}", tag="bqg", bufs=3)
                    nc.sync.dma_start(
                        out=_fr(bqg), in_=_fr(bqkv_d[:, mg * 512 : (mg + 1) * 512])
                    )
                    if mg < 4:
                        # q^T / k^T, transposed layout [m, s]
                        for ml in range(4):
                            m_abs = mg * 512 + ml * P
                            for j in range(2):
                                ps = psA.tile(
                                    [P, 512], F32, tag="pa", name=f"pa_{mg}_{ml}_{j}"
                                )
                                for n in range(8):
                                    nc.tensor.matmul(
                                        ps,
                                        _fr(wg[n][:, ml * P : (ml + 1) * P]),
                                        _fr(xT[n][:, j * 512 : (j + 1) * 512]),
                                        start=(n == 0),
                                        stop=False,
                                    )
                                nc.tensor.matmul(
                                    ps,
                                    _fr(bqg[:, ml * P : (ml + 1) * P]),
                                    _fr(ones_sb[:, j * 512 : (j + 1) * 512]),
                                    start=False,
                                    stop=True,
                                )
                                if m_abs < NX:
                                    # fold the 1/sqrt(D) scale into q
                                    nc.scalar.activation(
                                        _fr(qT[m_abs // P][:, j * 512 : (j + 1) * 512]),
                                        ps,
                                        AF.Copy,
                                        scale=0.125,
                                    )
                                else:
                                    ki = (m_abs - NX) // P
                                    nc.vector.tensor_copy(
                                        _fr(kT[ki][:, j * 512 : (j + 1) * 512]), ps
                                    )
                            if m_abs >= NX:
                                ki = (m_abs - NX) // P
                                nc.sync.dma_start(
                                    out=outk_d[ki * P : (ki + 1) * P, :], in_=kT[ki]
                                )
                    else:
                        # v, natural layout [t, hd] with ones column per head
                        mvg = mg - 4
                        h0 = mvg * 8
                        for t in range(8):
                            ps = psA.tile([P, 512], F32, tag="pa", name=f"pv_{mg}_{t}")
                            for n in range(8):
                                nc.tensor.matmul(
                                    ps,
                                    _fr(xT[n][:, t * P : (t + 1) * P]),
                                    _fr(wg[n]),
                                    start=(n == 0),
                                    stop=False,
                                )
                            nc.tensor.matmul(
                                ps,
                                _fr(ones_sb[:, t * P : (t + 1) * P]),
                                _fr(bqg),
                                start=False,
                                stop=True,
                            )
                            v3 = vpad[t].rearrange("p (h e) -> p h e", e=E)
                            nc.vector.tensor_copy(
                                _fr(v3[:, h0 : h0 + 8, 0:D]),
                                ps.rearrange("p (h d) -> p h d", d=D),
                            )
                            if mvg == 1:
                                nc.sync.dma_start(
                                    out=outv_d[t * P : (t + 1) * P, :].rearrange(
                                        "p (h d) -> p h d", d=D
                                    ),
                                    in_=v3[:, :, 0:D],
                                )

            # ---------------- attention ----------------
            with (
                tc.tile_pool(name="rcp", bufs=4) as rcp,
                tc.tile_pool(name="etp", bufs=9) as etp,
                tc.tile_pool(name="psT", bufs=2, space="PSUM") as psT,
            ):
                for h in range(H):
                    qc, qr = h // 2, (h % 2) * D
                    odd = h % 2 == 1
                    for j in range(2):
                        ntc = 4 * j + 4  # causal: t-chunks 0..4j+3
                        es = []
                        for c in range(ntc):
                            pss = psT.tile(
                                [P, 512],
                                F32,
                                tag="ps_s",
                                bufs=3,
                                name=f"pss_{h}_{j}_{c}",
                            )
                            nc.tensor.matmul(
                                pss,
                                _fr(kT[qc][qr : qr + D, c * P : (c + 1) * P]),
                                _fr(qT[qc][qr : qr + D, j * 512 : (j + 1) * 512]),
                                start=True,
                                stop=True,
                            )
                            off = c * P - j * 512
                            if off >= 0:  # diagonal block: full-width mask add
                                nc.vector.tensor_tensor(
                                    pss, pss, mask_sb[off // P], OP.add
                                )
                            e = etp.tile([P, 512], F32, tag="et", name=f"e_{h}_{j}_{c}")
                            nc.scalar.activation(_fr(e), pss, AF.Exp)
                            es.append(e)
                        # context rows 0..63 + denominator row 64, via the
                        # [v|1] fused stationary (matmul dst must sit at
                        # partition base 0)
                        psc = psT.tile(
                            [P, 512], F32, tag="ps_c", bufs=2, name=f"psc_{h}_{j}"
                        )
                        for c in range(ntc):
                            nc.tensor.matmul(
                                psc[0:E],
                                _fr(vpad[c][:, h * E : h * E + E]),
                                _fr(es[c]),
                                start=(c == 0),
                                stop=(c == ntc - 1),
                            )
                        rec = rcp.tile([P, 512], F32, tag="rec", name=f"rec_{h}_{j}")
                        rrow = rec[D : D + 1, :]
                        with nc.allow_low_precision(
                            reason="fp32r bitcast of an fp32 reciprocal row"
                        ):
                            nc.vector.reciprocal(_fr(rrow), psc[D : D + 1, :])
                        psb = psT.tile(
                            [P, 512], F32, tag="ps_b", bufs=2, name=f"psb_{h}_{j}"
                        )
                        nc.tensor.matmul(
                            psb[0:D],
                            _fr(ones64[D : D + 1, :]),
                            _fr(rrow),
                            start=True,
                            stop=True,
                        )
                        # DVE can read only one PSUM operand; stage the
                        # broadcast rows through SBUF
                        rb = rcp.tile([P, 512], F32, tag="rb", name=f"rb_{h}_{j}")
                        nc.scalar.activation(rb[0:D, :], psb[0:D], AF.Copy)
                        if not odd:
                            nc.vector.tensor_tensor(
                                _fr(aT[qc][0:D, j * 512 : (j + 1) * 512]),
                                psc[0:D, :],
                                rb[0:D, :],
                                OP.mult,
                            )
                        else:
                            # odd head result lives on partitions 64..127 of
                            # the pair tile; DVE is lane-bound, so normalize
                            # into a staging tile and move it with a DMA
                            tmp = rcp.tile([D, 512], F32, tag="tmp", name=f"tmp_{h}_{j}")
                            nc.vector.tensor_tensor(
                                _fr(tmp), psc[0:D, :], rb[0:D, :], OP.mult
                            )
                            nc.sync.dma_start(
                                out=_fr(aT[qc][D : D + D, j * 512 : (j + 1) * 512]),
                                in_=_fr(tmp),
                            )

            # ---------------- output projection ----------------
            with (
                tc.tile_pool(name="wpp", bufs=10) as wpp,
                tc.tile_pool(name="evp", bufs=4) as evp,
                tc.tile_pool(name="psP", bufs=4, space="PSUM") as psP,
            ):
                for mg in range(2):
                    wp = []
                    for n in range(8):
                        w = wpp.tile([P, 512], F32, name=f"wp_{mg}_{n}", tag="wp")
                        nc.sync.dma_start(
                            out=_fr(w),
                            in_=_fr(
                                wproj_d[n * P : (n + 1) * P, mg * 512 : (mg + 1) * 512]
                            ),
                        )
                        wp.append(w)
                    bpg = wpp.tile([1, 512], F32, name=f"bpg_{mg}", tag="bpg", bufs=2)
                    nc.sync.dma_start(
                        out=_fr(bpg), in_=_fr(bproj_d[:, mg * 512 : (mg + 1) * 512])
                    )
                    for i in range(8):
                        ps = psP.tile([P, 512], F32, tag="pp", name=f"pp_{mg}_{i}")
                        for n in range(8):
                            nc.tensor.matmul(
                                ps,
                                _fr(aT[n][:, i * P : (i + 1) * P]),
                                _fr(wp[n]),
                                start=(n == 0),
                                stop=False,
                            )
                        nc.tensor.matmul(
                            ps,
                            _fr(ones_sb[:, i * P : (i + 1) * P]),
                            _fr(bpg),
                            start=False,
                            stop=True,
                        )
                        ev = evp.tile([P, 512], F32, tag="ev", name=f"ev_{mg}_{i}")
                        nc.vector.tensor_copy(ev, ps)
                        nc.sync.dma_start(
                            out=outa_d[i * P : (i + 1) * P, mg * 512 : (mg + 1) * 512],
                            in_=ev,
                        )

    nc.compile()
    return nc


_NC_CACHE = None


def get_nc():
    global _NC_CACHE
    if _NC_CACHE is None:
        _NC_CACHE = build_nc()
    return _NC_CACHE


def make_in_maps(x, Wqkv, bqkv, Wproj, bproj):
    # cmask4[q][t, s]: additive mask for the diagonal-position-q score block
    # (psum block covers s columns q*128 aligned so that the diagonal sits at
    # columns [q*128, q*128+128))
    cols = np.arange(512)[None, :]
    rows = np.arange(P)[:, None]
    cmask4 = np.stack(
        [
            np.where(cols - q * P >= rows, 0.0, -10000.0).astype(np.float32)
            for q in range(4)
        ]
    )
    shared = {
        "Wqkv": np.ascontiguousarray(Wqkv, dtype=np.float32),
        "bqkv": np.ascontiguousarray(bqkv, dtype=np.float32).reshape(1, -1),
        "Wproj": np.ascontiguousarray(Wproj, dtype=np.float32),
        "bproj": np.ascontiguousarray(bproj, dtype=np.float32).reshape(1, -1),
        "cmask4": cmask4,
        "ones": np.ones((1, S), dtype=np.float32),
        "vones": np.ones((P, H), dtype=np.float32),
    }
    return [
        {"xT": np.ascontiguousarray(np.asarray(x[b], dtype=np.float32).T), **shared}
        for b in range(B)
    ]


def assemble_outputs(results):
    a = np.stack([r["out_a"] for r in results])  # [B, S, NX]
    k = np.stack(
        [r["out_kT"].reshape(H, D, S).transpose(0, 2, 1) for r in results]
    )  # [B, H, S, D]
    v = np.stack(
        [r["out_v"].reshape(S, H, D).transpose(1, 0, 2) for r in results]
    )  # [B, H, S, D]
    present = np.stack([k, v])  # [2, B, H, S, D]
    return a, present


def kernel(x, Wqkv, bqkv, Wproj, bproj):
    from concourse.bass_utils import run_bass_kernel_spmd

    nc = get_nc()
    in_maps = make_in_maps(x, Wqkv, bqkv, Wproj, bproj)
    res = run_bass_kernel_spmd(nc, in_maps, core_ids=list(range(B)))
    return assemble_outputs(res.results)


# revision 21
# speedup vs baseline: 1.1272x; 1.1272x over previous
"""Fused multi-head causal attention block (qkv proj + attention + out proj)
for Trainium2, data-parallel over batch across 8 NeuronCores.

Contract: kernel(**inputs) takes the FULL inputs
    x [8,1024,1024] f32, Wqkv [1024,3072], bqkv [3072], Wproj [1024,1024],
    bproj [1024]
and returns (a, present) exactly like the reference:
    a [8,1024,1024] f32, present [2,8,16,1024,64] f32.

Per-core program (SPMD, one batch element per core):
  stage A: qkv projection.  q^T,k^T produced transposed [hd, s] (head-major
           rows); v produced in natural [s, hd] layout padded with a ones
           column per head (width 65) so the context matmul can also yield
           the softmax denominator.
  attn:    scores computed transposed  w^T[t,s] = (k^T).T @ (q^T)  per head,
           causal blocks skipped, additive -1e4 mask on diagonal blocks,
           exp without max-subtraction (scores are O(1) here).  Context
           accumulated over t-chunks into one PSUM bank per (head, s-chunk):
           even heads write rows 0..64 via lhsT=[v|1] (row 64 = denominator),
           odd heads write rows 64..127 via lhsT=v plus a separate
           denominator group on row 0 of the same bank.  Normalize via a
           K=1 ones-matmul broadcast of 1/denom + DVE multiply, keeping every
           engine lane-aligned; head pairs share [128, S] aT tiles.
  proj:    out[s,m] = sum over 128-row hd chunks  aT[c].T @ Wproj[c]  + bias.

All matmul operands are bitcast to float32r (full PE rate at N=512).
Biases are applied exactly via K=1 rank-1 update matmuls.
"""

import numpy as np

import concourse.bass as bass
import concourse.mybir as mybir
import concourse.tile as tile
from concourse import bacc

B, S, NX, H, D = 8, 1024, 1024, 16, 64
P = 128
E = D + 1  # v columns per head incl. ones column
F32 = mybir.dt.float32
FR = mybir.dt.float32r
AF = mybir.ActivationFunctionType
OP = mybir.AluOpType


def _fr(ap):
    return ap.bitcast(FR)


def build_nc():
    nc = bacc.Bacc("TRN2", target_bir_lowering=False)

    xT_d = nc.dram_tensor("xT", [NX, S], F32, kind="ExternalInput")
    wqkv_d = nc.dram_tensor("Wqkv", [NX, 3 * NX], F32, kind="ExternalInput")
    bqkv_d = nc.dram_tensor("bqkv", [1, 3 * NX], F32, kind="ExternalInput")
    wproj_d = nc.dram_tensor("Wproj", [NX, NX], F32, kind="ExternalInput")
    bproj_d = nc.dram_tensor("bproj", [1, NX], F32, kind="ExternalInput")
    mask_d = nc.dram_tensor("cmask4", [4, P, 512], F32, kind="ExternalInput")
    ones_d = nc.dram_tensor("ones", [1, S], F32, kind="ExternalInput")
    vones_d = nc.dram_tensor("vones", [P, H], F32, kind="ExternalInput")
    rall_d = nc.dram_tensor("rall_scratch", [H, S], F32)
    outa_d = nc.dram_tensor("out_a", [S, NX], F32, kind="ExternalOutput")
    outk_d = nc.dram_tensor("out_kT", [NX, S], F32, kind="ExternalOutput")
    outv_d = nc.dram_tensor("out_v", [S, NX], F32, kind="ExternalOutput")

    with tile.TileContext(nc) as tc:
        with (
            tc.tile_pool(name="const", bufs=1) as constp,
            tc.tile_pool(name="qk", bufs=1, side="right") as qkp,
            tc.tile_pool(name="vpp", bufs=1, side="right") as vpp,
            tc.tile_pool(name="atp", bufs=1) as atp,
        ):
            # Memset cannot produce fp32r-rounded data, so all-ones constants
            # consumed by matmuls are DMA'd from DRAM inputs instead.
            ones_sb = constp.tile([1, S], F32, name="ones_sb")
            nc.sync.dma_start(out=_fr(ones_sb), in_=_fr(ones_d[:, :]))
            # full-width additive causal masks, one per diagonal position:
            # cols < q*128 get -1e4 (fully masked), the 128-wide diagonal
            # block gets the triangular mask, cols beyond it get 0
            # only the first (q+1)*128 columns of position-q's mask are
            # nonzero-interesting; columns right of the diagonal stay unmasked
            mask_sb = [
                constp.tile(
                    [P, (q + 1) * P], F32, name=f"mask_sb{q}", tag=f"mask_sb{q}"
                )
                for q in range(4)
            ]
            for q in range(4):
                nc.sync.dma_start(out=mask_sb[q], in_=mask_d[q, :, 0 : (q + 1) * P])

            qT = [qkp.tile([P, S], F32, name=f"qT{i}", tag=f"qT{i}") for i in range(8)]
            kT = [qkp.tile([P, S], F32, name=f"kT{i}", tag=f"kT{i}") for i in range(8)]
            vpad = [
                vpp.tile([P, H * E], F32, name=f"vp{i}", tag=f"vp{i}") for i in range(8)
            ]
            # head pair c: head 2c on partitions 0..63, head 2c+1 on 64..127
            aT = [atp.tile([P, S], F32, name=f"aT{c}", tag=f"aT{c}") for c in range(8)]

            # ---------------- stage A: qkv = x @ Wqkv + b ----------------
            with (
                tc.tile_pool(name="xp", bufs=1) as xp,
                tc.tile_pool(name="wgp", bufs=10) as wgp,
                tc.tile_pool(name="psA", bufs=5, space="PSUM") as psA,
            ):
                xT = []
                for n in range(8):
                    t = xp.tile([P, S], F32, name=f"xT{n}", tag=f"xT{n}")
                    nc.sync.dma_start(out=_fr(t), in_=_fr(xT_d[n * P : (n + 1) * P, :]))
                    xT.append(t)
                for t in range(8):
                    v3 = vpad[t].rearrange("p (h e) -> p h e", e=E)
                    nc.sync.dma_start(
                        out=_fr(v3[:, :, D : D + 1]), in_=_fr(vones_d[:, :, None])
                    )

                for mg in range(6):
                    wg = []
                    for n in range(8):
                        w = wgp.tile([P, 512], F32, name=f"wg_{mg}_{n}", tag="wg")
                        nc.sync.dma_start(
                            out=_fr(w),
                            in_=_fr(
                                wqkv_d[n * P : (n + 1) * P, mg * 512 : (mg + 1) * 512]
                            ),
                        )
                        wg.append(w)
                    bqg = wgp.tile([1, 512], F32, name=f"bqg_{mg}", tag="bqg", bufs=3)
                    nc.sync.dma_start(
                        out=_fr(bqg), in_=_fr(bqkv_d[:, mg * 512 : (mg + 1) * 512])
                    )
                    if mg < 4:
                        # q^T / k^T, transposed layout [m, s]
                        for ml in range(4):
                            m_abs = mg * 512 + ml * P
                            for j in range(2):
                                ps = psA.tile(
                                    [P, 512], F32, tag="pa", name=f"pa_{mg}_{ml}_{j}"
                                )
                                for n in range(8):
                                    nc.tensor.matmul(
                                        ps,
                                        _fr(wg[n][:, ml * P : (ml + 1) * P]),
                                        _fr(xT[n][:, j * 512 : (j + 1) * 512]),
                                        start=(n == 0),
                                        stop=False,
                                    )
                                nc.tensor.matmul(
                                    ps,
                                    _fr(bqg[:, ml * P : (ml + 1) * P]),
                                    _fr(ones_sb[:, j * 512 : (j + 1) * 512]),
                                    start=False,
                                    stop=True,
                                )
                                if m_abs < NX:
                                    # fold the 1/sqrt(D) scale into q
                                    nc.scalar.activation(
                                        _fr(qT[m_abs // P][:, j * 512 : (j + 1) * 512]),
                                        ps,
                                        AF.Copy,
                                        scale=0.125,
                                    )
                                else:
                                    ki = (m_abs - NX) // P
                                    nc.vector.tensor_copy(
                                        _fr(kT[ki][:, j * 512 : (j + 1) * 512]), ps
                                    )
                            if m_abs >= NX:
                                ki = (m_abs - NX) // P
                                nc.sync.dma_start(
                                    out=outk_d[ki * P : (ki + 1) * P, :], in_=kT[ki]
                                )
                    else:
                        # v, natural layout [t, hd] with ones column per head
                        mvg = mg - 4
                        h0 = mvg * 8
                        for t in range(8):
                            ps = psA.tile([P, 512], F32, tag="pa", name=f"pv_{mg}_{t}")
                            for n in range(8):
                                nc.tensor.matmul(
                                    ps,
                                    _fr(xT[n][:, t * P : (t + 1) * P]),
                                    _fr(wg[n]),
                                    start=(n == 0),
                                    stop=False,
                                )
                            nc.tensor.matmul(
                                ps,
                                _fr(ones_sb[:, t * P : (t + 1) * P]),
                                _fr(bqg),
                                start=False,
                                stop=True,
                            )
                            v3 = vpad[t].rearrange("p (h e) -> p h e", e=E)
                            nc.vector.tensor_copy(
                                _fr(v3[:, h0 : h0 + 8, 0:D]),
                                ps.rearrange("p (h d) -> p h d", d=D),
                            )
                            if mvg == 1:
                                nc.sync.dma_start(
                                    out=outv_d[t * P : (t + 1) * P, :].rearrange(
                                        "p (h d) -> p h d", d=D
                                    ),
                                    in_=v3[:, :, 0:D],
                                )

            # ---------------- attention ----------------
            with (
                tc.tile_pool(name="rcp", bufs=4) as rcp,
                tc.tile_pool(name="etp", bufs=12) as etp,
                tc.tile_pool(name="psT", bufs=2, space="PSUM") as psT,
            ):
                # all 32 softmax denominators collect here (row = head), so a
                # single batched reciprocal replaces 32 serial one-lane ones
                den_all = atp.tile([H, S], F32, name="den_all")
                rall = atp.tile([H, S], F32, name="rall")
                for h in range(H):
                    qc, qr = h // 2, (h % 2) * D
                    odd = h % 2 == 1
                    for j in range(2):
                        ntc = 4 * j + 4  # causal: t-chunks 0..4j+3
                        es = []
                        for c in range(ntc):
                            pss = psT.tile(
                                [P, 512],
                                F32,
                                tag="ps_s",
                                bufs=3,
                                name=f"pss_{h}_{j}_{c}",
                            )
                            nc.tensor.matmul(
                                pss,
                                _fr(kT[qc][qr : qr + D, c * P : (c + 1) * P]),
                                _fr(qT[qc][qr : qr + D, j * 512 : (j + 1) * 512]),
                                start=True,
                                stop=True,
                            )
                            off = c * P - j * 512
                            if off >= 0:  # diagonal block: mask add
                                w = off + P
                                nc.vector.tensor_tensor(
                                    pss[:, 0:w], pss[:, 0:w], mask_sb[off // P], OP.add
                                )
                            e = etp.tile([P, 512], F32, tag="et", name=f"e_{h}_{j}_{c}")
                            nc.scalar.activation(_fr(e), pss, AF.Exp)
                            es.append(e)
                        # context rows 0..63 + denominator row 64, via the
                        # [v|1] fused stationary (matmul dst must sit at
                        # partition base 0)
                        psc = psT.tile(
                            [P, 512], F32, tag="ps_c", bufs=2, name=f"psc_{h}_{j}"
                        )
                        for c in range(ntc):
                            nc.tensor.matmul(
                                psc[0:E],
                                _fr(vpad[c][:, h * E : h * E + E]),
                                _fr(es[c]),
                                start=(c == 0),
                                stop=(c == ntc - 1),
                            )
                        # stash the denominator row (ACT: PSUM row 64 -> SBUF
                        # row 64, then a tiny DMA to partition h of den_all) —
                        # keeps the reciprocal OFF the PE critical path
                        rec = rcp.tile([P, 512], F32, tag="rec", name=f"rec_{h}_{j}")
                        nc.scalar.activation(
                            rec[D : D + 1, :], psc[D : D + 1, :], AF.Copy
                        )
                        nc.sync.dma_start(
                            out=den_all[h : h + 1, j * 512 : (j + 1) * 512],
                            in_=rec[D : D + 1, :],
                        )
                        # evict the unnormalized context rows
                        if not odd:
                            nc.vector.tensor_copy(
                                _fr(aT[qc][0:D, j * 512 : (j + 1) * 512]),
                                psc[0:D, :],
                            )
                        else:
                            # odd head rows live on partitions 64..127 of the
                            # pair tile; DVE is lane-bound, so stage + DMA
                            tmp = rcp.tile([D, 512], F32, tag="tmp", name=f"tmp_{h}_{j}")
                            nc.vector.tensor_copy(_fr(tmp), psc[0:D, :])
                            nc.sync.dma_start(
                                out=_fr(aT[qc][D : D + D, j * 512 : (j + 1) * 512]),
                                in_=_fr(tmp),
                            )
                # one batched reciprocal over all (head, s) denominators
                nc.vector.reciprocal(rall, den_all)
                # bounce the recips through DRAM so partition-broadcast DMAs
                # can fan each head's row out across its 64 partitions
                nc.sync.dma_start(out=rall_d[:, :], in_=rall)
                for c8 in range(8):
                    for j in range(2):
                        rb = rcp.tile([P, 512], F32, tag="rb", name=f"rb_{c8}_{j}")
                        for r in range(2):
                            row = rall_d[2 * c8 + r, j * 512 : (j + 1) * 512]
                            nc.gpsimd.dma_start(
                                out=rb[r * D : (r + 1) * D, :],
                                in_=row[None, :].to_broadcast((D, 512)),
                            )
                        nc.vector.tensor_tensor(
                            _fr(aT[c8][:, j * 512 : (j + 1) * 512]),
                            aT[c8][:, j * 512 : (j + 1) * 512],
                            rb,
                            OP.mult,
                        )

            # ---------------- output projection ----------------
            with (
                tc.tile_pool(name="wpp", bufs=10) as wpp,
                tc.tile_pool(name="evp", bufs=4) as evp,
                tc.tile_pool(name="psP", bufs=4, space="PSUM") as psP,
            ):
                for mg in range(2):
                    wp = []
                    for n in range(8):
                        w = wpp.tile([P, 512], F32, name=f"wp_{mg}_{n}", tag="wp")
                        nc.sync.dma_start(
                            out=_fr(w),
                            in_=_fr(
                                wproj_d[n * P : (n + 1) * P, mg * 512 : (mg + 1) * 512]
                            ),
                        )
                        wp.append(w)
                    bpg = wpp.tile([1, 512], F32, name=f"bpg_{mg}", tag="bpg", bufs=2)
                    nc.sync.dma_start(
                        out=_fr(bpg), in_=_fr(bproj_d[:, mg * 512 : (mg + 1) * 512])
                    )
                    for i in range(8):
                        ps = psP.tile([P, 512], F32, tag="pp", name=f"pp_{mg}_{i}")
                        for n in range(8):
                            nc.tensor.matmul(
                                ps,
                                _fr(aT[n][:, i * P : (i + 1) * P]),
                                _fr(wp[n]),
                                start=(n == 0),
                                stop=False,
                            )
                        nc.tensor.matmul(
                            ps,
                            _fr(ones_sb[:, i * P : (i + 1) * P]),
                            _fr(bpg),
                            start=False,
                            stop=True,
                        )
                        ev = evp.tile([P, 512], F32, tag="ev", name=f"ev_{mg}_{i}")
                        nc.vector.tensor_copy(ev, ps)
                        nc.sync.dma_start(
                            out=outa_d[i * P : (i + 1) * P, mg * 512 : (mg + 1) * 512],
                            in_=ev,
                        )

    nc.compile()
    return nc


_NC_CACHE = None


def get_nc():
    global _NC_CACHE
    if _NC_CACHE is None:
        _NC_CACHE = build_nc()
    return _NC_CACHE


def make_in_maps(x, Wqkv, bqkv, Wproj, bproj):
    # cmask4[q][t, s]: additive mask for the diagonal-position-q score block
    # (psum block covers s columns q*128 aligned so that the diagonal sits at
    # columns [q*128, q*128+128))
    cols = np.arange(512)[None, :]
    rows = np.arange(P)[:, None]
    cmask4 = np.stack(
        [
            np.where(cols - q * P >= rows, 0.0, -10000.0).astype(np.float32)
            for q in range(4)
        ]
    )
    shared = {
        "Wqkv": np.ascontiguousarray(Wqkv, dtype=np.float32),
        "bqkv": np.ascontiguousarray(bqkv, dtype=np.float32).reshape(1, -1),
        "Wproj": np.ascontiguousarray(Wproj, dtype=np.float32),
        "bproj": np.ascontiguousarray(bproj, dtype=np.float32).reshape(1, -1),
        "cmask4": cmask4,
        "ones": np.ones((1, S), dtype=np.float32),
        "vones": np.ones((P, H), dtype=np.float32),
    }
    return [
        {"xT": np.ascontiguousarray(np.asarray(x[b], dtype=np.float32).T), **shared}
        for b in range(B)
    ]


def assemble_outputs(results):
    a = np.stack([r["out_a"] for r in results])  # [B, S, NX]
    k = np.stack(
        [r["out_kT"].reshape(H, D, S).transpose(0, 2, 1) for r in results]
    )  # [B, H, S, D]
    v = np.stack(
        [r["out_v"].reshape(S, H, D).transpose(1, 0, 2) for r in results]
    )  # [B, H, S, D]
    present = np.stack([k, v])  # [2, B, H, S, D]
    return a, present


def kernel(x, Wqkv, bqkv, Wproj, bproj):
    from concourse.bass_utils import run_bass_kernel_spmd

    nc = get_nc()
    in_maps = make_in_maps(x, Wqkv, bqkv, Wproj, bproj)
    res = run_bass_kernel_spmd(nc, in_maps, core_ids=list(range(B)))
    return assemble_outputs(res.results)


# revision 23
# speedup vs baseline: 1.1326x; 1.0048x over previous
"""Fused multi-head causal attention block (qkv proj + attention + out proj)
for Trainium2, data-parallel over batch across 8 NeuronCores.

Contract: kernel(**inputs) takes the FULL inputs
    x [8,1024,1024] f32, Wqkv [1024,3072], bqkv [3072], Wproj [1024,1024],
    bproj [1024]
and returns (a, present) exactly like the reference:
    a [8,1024,1024] f32, present [2,8,16,1024,64] f32.

Per-core program (SPMD, one batch element per core):
  stage A: qkv projection.  q^T,k^T produced transposed [hd, s] (head-major
           rows); v produced in natural [s, hd] layout padded with a ones
           column per head (width 65) so the context matmul can also yield
           the softmax denominator.
  attn:    scores computed transposed  w^T[t,s] = (k^T).T @ (q^T)  per head,
           causal blocks skipped, additive -1e4 mask on diagonal blocks,
           exp without max-subtraction (scores are O(1) here).  Context
           accumulated over t-chunks into one PSUM bank per (head, s-chunk):
           even heads write rows 0..64 via lhsT=[v|1] (row 64 = denominator),
           odd heads write rows 64..127 via lhsT=v plus a separate
           denominator group on row 0 of the same bank.  Normalize via a
           K=1 ones-matmul broadcast of 1/denom + DVE multiply, keeping every
           engine lane-aligned; head pairs share [128, S] aT tiles.
  proj:    out[s,m] = sum over 128-row hd chunks  aT[c].T @ Wproj[c]  + bias.

All matmul operands are bitcast to float32r (full PE rate at N=512).
Biases are applied exactly via K=1 rank-1 update matmuls.
"""

import numpy as np

import concourse.bass as bass
import concourse.mybir as mybir
import concourse.tile as tile
from concourse import bacc

B, S, NX, H, D = 8, 1024, 1024, 16, 64
P = 128
E = D + 1  # v columns per head incl. ones column
F32 = mybir.dt.float32
FR = mybir.dt.float32r
AF = mybir.ActivationFunctionType
OP = mybir.AluOpType


def _fr(ap):
    return ap.bitcast(FR)


def build_nc():
    nc = bacc.Bacc("TRN2", target_bir_lowering=False)

    xT_d = nc.dram_tensor("xT", [NX, S], F32, kind="ExternalInput")
    wqkv_d = nc.dram_tensor("Wqkv", [NX, 3 * NX], F32, kind="ExternalInput")
    bqkv_d = nc.dram_tensor("bqkv", [1, 3 * NX], F32, kind="ExternalInput")
    wproj_d = nc.dram_tensor("Wproj", [NX, NX], F32, kind="ExternalInput")
    bproj_d = nc.dram_tensor("bproj", [1, NX], F32, kind="ExternalInput")
    mask_d = nc.dram_tensor("cmask4", [4, P, 512], F32, kind="ExternalInput")
    ones_d = nc.dram_tensor("ones", [1, S], F32, kind="ExternalInput")
    vones_d = nc.dram_tensor("vones", [P, H], F32, kind="ExternalInput")
    sel8_d = nc.dram_tensor("sel8", [8, H, P], F32, kind="ExternalInput")
    outa_d = nc.dram_tensor("out_a", [S, NX], F32, kind="ExternalOutput")
    outk_d = nc.dram_tensor("out_kT", [NX, S], F32, kind="ExternalOutput")
    outv_d = nc.dram_tensor("out_v", [S, NX], F32, kind="ExternalOutput")

    with tile.TileContext(nc) as tc:
        with (
            tc.tile_pool(name="const", bufs=1) as constp,
            tc.tile_pool(name="qk", bufs=1, side="right") as qkp,
            tc.tile_pool(name="vpp", bufs=1, side="right") as vpp,
            tc.tile_pool(name="atp", bufs=1) as atp,
        ):
            # Memset cannot produce fp32r-rounded data, so all-ones constants
            # consumed by matmuls are DMA'd from DRAM inputs instead.
            ones_sb = constp.tile([1, S], F32, name="ones_sb")
            nc.sync.dma_start(out=_fr(ones_sb), in_=_fr(ones_d[:, :]))
            # full-width additive causal masks, one per diagonal position:
            # cols < q*128 get -1e4 (fully masked), the 128-wide diagonal
            # block gets the triangular mask, cols beyond it get 0
            # per-pair normalize selectors: sel8[c8][r, p] = 1 iff
            # r == 2*c8 + (p >= 64), so sel8[c8].T @ rall broadcasts the
            # pair's two recip rows onto its 128 partitions
            sel8_sb = [
                constp.tile([H, P], F32, name=f"sel8_{c}", tag=f"sel8_{c}")
                for c in range(8)
            ]
            for c in range(8):
                nc.sync.dma_start(out=_fr(sel8_sb[c]), in_=_fr(sel8_d[c]))
            # only the first (q+1)*128 columns of position-q's mask are
            # nonzero-interesting; columns right of the diagonal stay unmasked
            mask_sb = [
                constp.tile(
                    [P, (q + 1) * P], F32, name=f"mask_sb{q}", tag=f"mask_sb{q}"
                )
                for q in range(4)
            ]
            for q in range(4):
                nc.sync.dma_start(out=mask_sb[q], in_=mask_d[q, :, 0 : (q + 1) * P])

            qT = [qkp.tile([P, S], F32, name=f"qT{i}", tag=f"qT{i}") for i in range(8)]
            kT = [qkp.tile([P, S], F32, name=f"kT{i}", tag=f"kT{i}") for i in range(8)]
            vpad = [
                vpp.tile([P, H * E], F32, name=f"vp{i}", tag=f"vp{i}") for i in range(8)
            ]
            # head pair c: head 2c on partitions 0..63, head 2c+1 on 64..127
            aT = [atp.tile([P, S], F32, name=f"aT{c}", tag=f"aT{c}") for c in range(8)]

            # ---------------- stage A: qkv = x @ Wqkv + b ----------------
            with (
                tc.tile_pool(name="xp", bufs=1) as xp,
                tc.tile_pool(name="wgp", bufs=10) as wgp,
                tc.tile_pool(name="psA", bufs=5, space="PSUM") as psA,
            ):
                xT = []
                for n in range(8):
                    t = xp.tile([P, S], F32, name=f"xT{n}", tag=f"xT{n}")
                    nc.sync.dma_start(out=_fr(t), in_=_fr(xT_d[n * P : (n + 1) * P, :]))
                    xT.append(t)
                for t in range(8):
                    v3 = vpad[t].rearrange("p (h e) -> p h e", e=E)
                    nc.sync.dma_start(
                        out=_fr(v3[:, :, D : D + 1]), in_=_fr(vones_d[:, :, None])
                    )

                for mg in range(6):
                    wg = []
                    for n in range(8):
                        w = wgp.tile([P, 512], F32, name=f"wg_{mg}_{n}", tag="wg")
                        nc.sync.dma_start(
                            out=_fr(w),
                            in_=_fr(
                                wqkv_d[n * P : (n + 1) * P, mg * 512 : (mg + 1) * 512]
                            ),
                        )
                        wg.append(w)
                    bqg = wgp.tile([1, 512], F32, name=f"bqg_{mg}", tag="bqg", bufs=3)
                    nc.sync.dma_start(
                        out=_fr(bqg), in_=_fr(bqkv_d[:, mg * 512 : (mg + 1) * 512])
                    )
                    if mg < 4:
                        # q^T / k^T, transposed layout [m, s]
                        for ml in range(4):
                            m_abs = mg * 512 + ml * P
                            for j in range(2):
                                ps = psA.tile(
                                    [P, 512], F32, tag="pa", name=f"pa_{mg}_{ml}_{j}"
                                )
                                for n in range(8):
                                    nc.tensor.matmul(
                                        ps,
                                        _fr(wg[n][:, ml * P : (ml + 1) * P]),
                                        _fr(xT[n][:, j * 512 : (j + 1) * 512]),
                                        start=(n == 0),
                                        stop=False,
                                    )
                                nc.tensor.matmul(
                                    ps,
                                    _fr(bqg[:, ml * P : (ml + 1) * P]),
                                    _fr(ones_sb[:, j * 512 : (j + 1) * 512]),
                                    start=False,
                                    stop=True,
                                )
                                if m_abs < NX:
                                    # fold the 1/sqrt(D) scale into q
                                    nc.scalar.activation(
                                        _fr(qT[m_abs // P][:, j * 512 : (j + 1) * 512]),
                                        ps,
                                        AF.Copy,
                                        scale=0.125,
                                    )
                                else:
                                    ki = (m_abs - NX) // P
                                    nc.vector.tensor_copy(
                                        _fr(kT[ki][:, j * 512 : (j + 1) * 512]), ps
                                    )
                            if m_abs >= NX:
                                ki = (m_abs - NX) // P
                                nc.sync.dma_start(
                                    out=outk_d[ki * P : (ki + 1) * P, :], in_=kT[ki]
                                )
                    else:
                        # v, natural layout [t, hd] with ones column per head
                        mvg = mg - 4
                        h0 = mvg * 8
                        for t in range(8):
                            ps = psA.tile([P, 512], F32, tag="pa", name=f"pv_{mg}_{t}")
                            for n in range(8):
                                nc.tensor.matmul(
                                    ps,
                                    _fr(xT[n][:, t * P : (t + 1) * P]),
                                    _fr(wg[n]),
                                    start=(n == 0),
                                    stop=False,
                                )
                            nc.tensor.matmul(
                                ps,
                                _fr(ones_sb[:, t * P : (t + 1) * P]),
                                _fr(bqg),
                                start=False,
                                stop=True,
                            )
                            v3 = vpad[t].rearrange("p (h e) -> p h e", e=E)
                            nc.vector.tensor_copy(
                                _fr(v3[:, h0 : h0 + 8, 0:D]),
                                ps.rearrange("p (h d) -> p h d", d=D),
                            )
                            if mvg == 1:
                                nc.sync.dma_start(
                                    out=outv_d[t * P : (t + 1) * P, :].rearrange(
                                        "p (h d) -> p h d", d=D
                                    ),
                                    in_=v3[:, :, 0:D],
                                )

            # ---------------- attention ----------------
            with (
                tc.tile_pool(name="rcp", bufs=4) as rcp,
                tc.tile_pool(name="etp", bufs=12) as etp,
                tc.tile_pool(name="psT", bufs=2, space="PSUM") as psT,
            ):
                # all 32 softmax denominators collect here (row = head), so a
                # single batched reciprocal replaces 32 serial one-lane ones
                den_all = atp.tile([H, S], F32, name="den_all")
                rall = atp.tile([H, S], F32, name="rall")
                # Head PAIRS are processed together: the even head's score
                # matmuls use PE rows 0..63 and the odd head's rows 64..127
                # (disjoint row groups), so adjacent even/odd matmuls run
                # CONCURRENTLY on the PE and keep the whole array active
                # (half-array activity lets the HAM clock-gate throttle).
                for c8 in range(8):
                    for j in range(2):
                        ntc = 4 * j + 4  # causal: t-chunks 0..4j+3
                        es = {0: [], 1: []}
                        for c in range(ntc):
                            pss = {}
                            for r in range(2):
                                qr = r * D
                                pss[r] = psT.tile(
                                    [P, 512],
                                    F32,
                                    tag=f"ps_s{r}",
                                    bufs=2,
                                    name=f"pss_{c8}_{j}_{c}_{r}",
                                )
                                nc.tensor.matmul(
                                    pss[r],
                                    _fr(kT[c8][qr : qr + D, c * P : (c + 1) * P]),
                                    _fr(qT[c8][qr : qr + D, j * 512 : (j + 1) * 512]),
                                    start=True,
                                    stop=True,
                                )
                            off = c * P - j * 512
                            for r in range(2):
                                if off >= 0:  # diagonal block: mask add
                                    w = off + P
                                    nc.vector.tensor_tensor(
                                        pss[r][:, 0:w],
                                        pss[r][:, 0:w],
                                        mask_sb[off // P],
                                        OP.add,
                                    )
                                e = etp.tile(
                                    [P, 512], F32, tag="et", name=f"e_{c8}_{j}_{c}_{r}"
                                )
                                nc.scalar.activation(_fr(e), pss[r], AF.Exp)
                                es[r].append(e)
                        # context rows 0..63 + denominator row 64, via the
                        # [v|1] fused stationary (matmul dst at partition 0)
                        psc = {}
                        for r in range(2):
                            h = 2 * c8 + r
                            psc[r] = psT.tile(
                                [P, 512], F32, tag=f"ps_c{r}", bufs=2,
                                name=f"psc_{c8}_{j}_{r}",
                            )
                            for c in range(ntc):
                                nc.tensor.matmul(
                                    psc[r][0:E],
                                    _fr(vpad[c][:, h * E : h * E + E]),
                                    _fr(es[r][c]),
                                    start=(c == 0),
                                    stop=(c == ntc - 1),
                                )
                        for r in range(2):
                            h = 2 * c8 + r
                            # stash the denominator row (ACT: PSUM row 64 ->
                            # SBUF row 64, then a tiny DMA to partition h of
                            # den_all) — keeps reciprocal off the PE path
                            rec = rcp.tile([P, 512], F32, tag="rec", name=f"rec_{h}_{j}")
                            nc.scalar.activation(
                                rec[D : D + 1, :], psc[r][D : D + 1, :], AF.Copy
                            )
                            nc.sync.dma_start(
                                out=den_all[h : h + 1, j * 512 : (j + 1) * 512],
                                in_=rec[D : D + 1, :],
                            )
                            # evict the unnormalized context rows
                            if r == 0:
                                nc.vector.tensor_copy(
                                    _fr(aT[c8][0:D, j * 512 : (j + 1) * 512]),
                                    psc[r][0:D, :],
                                )
                            else:
                                # odd head rows live on partitions 64..127 of
                                # the pair tile; DVE is lane-bound: stage + DMA
                                tmp = rcp.tile(
                                    [D, 512], F32, tag="tmp", name=f"tmp_{h}_{j}"
                                )
                                nc.vector.tensor_copy(_fr(tmp), psc[r][0:D, :])
                                nc.sync.dma_start(
                                    out=_fr(aT[c8][D : D + D, j * 512 : (j + 1) * 512]),
                                    in_=_fr(tmp),
                                )
                # one batched reciprocal over all (head, s) denominators
                with nc.allow_low_precision(
                    reason="fp32r rounding of softmax reciprocal rows"
                ):
                    nc.vector.reciprocal(_fr(rall), den_all)

            # normalize: K=16 selector matmul broadcasts each pair's two
            # recip rows across its 128 partitions, then one DVE multiply
            with tc.tile_pool(name="psN", bufs=2, space="PSUM") as psN:
                for c8 in range(8):
                    for j in range(2):
                        psb = psN.tile(
                            [P, 512], F32, tag="ps_b", bufs=2, name=f"psb_{c8}_{j}"
                        )
                        nc.tensor.matmul(
                            psb,
                            _fr(sel8_sb[c8]),
                            _fr(rall[:, j * 512 : (j + 1) * 512]),
                            start=True,
                            stop=True,
                        )
                        nc.vector.tensor_tensor(
                            _fr(aT[c8][:, j * 512 : (j + 1) * 512]),
                            aT[c8][:, j * 512 : (j + 1) * 512],
                            psb,
                            OP.mult,
                        )

            # ---------------- output projection ----------------
            with (
                tc.tile_pool(name="wpp", bufs=10) as wpp,
                tc.tile_pool(name="evp", bufs=4) as evp,
                tc.tile_pool(name="psP", bufs=4, space="PSUM") as psP,
            ):
                for mg in range(2):
                    wp = []
                    for n in range(8):
                        w = wpp.tile([P, 512], F32, name=f"wp_{mg}_{n}", tag="wp")
                        nc.sync.dma_start(
                            out=_fr(w),
                            in_=_fr(
                                wproj_d[n * P : (n + 1) * P, mg * 512 : (mg + 1) * 512]
                            ),
                        )
                        wp.append(w)
                    bpg = wpp.tile([1, 512], F32, name=f"bpg_{mg}", tag="bpg", bufs=2)
                    nc.sync.dma_start(
                        out=_fr(bpg), in_=_fr(bproj_d[:, mg * 512 : (mg + 1) * 512])
                    )
                    for i in range(8):
                        ps = psP.tile([P, 512], F32, tag="pp", name=f"pp_{mg}_{i}")
                        for n in range(8):
                            nc.tensor.matmul(
                                ps,
                                _fr(aT[n][:, i * P : (i + 1) * P]),
                                _fr(wp[n]),
                                start=(n == 0),
                                stop=False,
                            )
                        nc.tensor.matmul(
                            ps,
                            _fr(ones_sb[:, i * P : (i + 1) * P]),
                            _fr(bpg),
                            start=False,
                            stop=True,
                        )
                        ev = evp.tile([P, 512], F32, tag="ev", name=f"ev_{mg}_{i}")
                        nc.vector.tensor_copy(ev, ps)
                        nc.sync.dma_start(
                            out=outa_d[i * P : (i + 1) * P, mg * 512 : (mg + 1) * 512],
                            in_=ev,
                        )

    nc.compile()
    return nc


_NC_CACHE = None


def get_nc():
    global _NC_CACHE
    if _NC_CACHE is None:
        _NC_CACHE = build_nc()
    return _NC_CACHE


def make_in_maps(x, Wqkv, bqkv, Wproj, bproj):
    # cmask4[q][t, s]: additive mask for the diagonal-position-q score block
    # (psum block covers s columns q*128 aligned so that the diagonal sits at
    # columns [q*128, q*128+128))
    cols = np.arange(512)[None, :]
    rows = np.arange(P)[:, None]
    cmask4 = np.stack(
        [
            np.where(cols - q * P >= rows, 0.0, -10000.0).astype(np.float32)
            for q in range(4)
        ]
    )
    shared = {
        "Wqkv": np.ascontiguousarray(Wqkv, dtype=np.float32),
        "bqkv": np.ascontiguousarray(bqkv, dtype=np.float32).reshape(1, -1),
        "Wproj": np.ascontiguousarray(Wproj, dtype=np.float32),
        "bproj": np.ascontiguousarray(bproj, dtype=np.float32).reshape(1, -1),
        "cmask4": cmask4,
        "ones": np.ones((1, S), dtype=np.float32),
        "vones": np.ones((P, H), dtype=np.float32),
        "sel8": np.stack(
            [
                np.stack(
                    [
                        (np.full(P, r) == 2 * c + (np.arange(P) >= D)).astype(
                            np.float32
                        )
                        for r in range(H)
                    ]
                )
                for c in range(8)
            ]
        ),
    }
    return [
        {"xT": np.ascontiguousarray(np.asarray(x[b], dtype=np.float32).T), **shared}
        for b in range(B)
    ]


def assemble_outputs(results):
    a = np.stack([r["out_a"] for r in results])  # [B, S, NX]
    k = np.stack(
        [r["out_kT"].reshape(H, D, S).transpose(0, 2, 1) for r in results]
    )  # [B, H, S, D]
    v = np.stack(
        [r["out_v"].reshape(S, H, D).transpose(1, 0, 2) for r in results]
    )  # [B, H, S, D]
    present = np.stack([k, v])  # [2, B, H, S, D]
    return a, present


def kernel(x, Wqkv, bqkv, Wproj, bproj):
    from concourse.bass_utils import run_bass_kernel_spmd

    nc = get_nc()
    in_maps = make_in_maps(x, Wqkv, bqkv, Wproj, bproj)
    res = run_bass_kernel_spmd(nc, in_maps, core_ids=list(range(B)))
    return assemble_outputs(res.results)


# revision 25
# speedup vs baseline: 1.1668x; 1.0302x over previous
"""Fused multi-head causal attention block (qkv proj + attention + out proj)
for Trainium2, data-parallel over batch across 8 NeuronCores.

Contract: kernel(**inputs) takes the FULL inputs
    x [8,1024,1024] f32, Wqkv [1024,3072], bqkv [3072], Wproj [1024,1024],
    bproj [1024]
and returns (a, present) exactly like the reference:
    a [8,1024,1024] f32, present [2,8,16,1024,64] f32.

Per-core program (SPMD, one batch element per core):
  stage A: qkv projection.  q^T,k^T produced transposed [hd, s] (head-major
           rows); v produced in natural [s, hd] layout padded with a ones
           column per head (width 65) so the context matmul can also yield
           the softmax denominator.
  attn:    scores computed transposed  w^T[t,s] = (k^T).T @ (q^T)  per head,
           causal blocks skipped, additive -1e4 mask on diagonal blocks,
           exp without max-subtraction (scores are O(1) here).  Context
           accumulated over t-chunks into one PSUM bank per (head, s-chunk):
           even heads write rows 0..64 via lhsT=[v|1] (row 64 = denominator),
           odd heads write rows 64..127 via lhsT=v plus a separate
           denominator group on row 0 of the same bank.  Normalize via a
           K=1 ones-matmul broadcast of 1/denom + DVE multiply, keeping every
           engine lane-aligned; head pairs share [128, S] aT tiles.
  proj:    out[s,m] = sum over 128-row hd chunks  aT[c].T @ Wproj[c]  + bias.

All matmul operands are bitcast to float32r (full PE rate at N=512).
Biases are applied exactly via K=1 rank-1 update matmuls.
"""

import numpy as np

import concourse.bass as bass
import concourse.mybir as mybir
import concourse.tile as tile
from concourse import bacc

B, S, NX, H, D = 8, 1024, 1024, 16, 64
P = 128
E = D + 1  # v columns per head incl. ones column
F32 = mybir.dt.float32
FR = mybir.dt.float32r
AF = mybir.ActivationFunctionType
OP = mybir.AluOpType


def _fr(ap):
    return ap.bitcast(FR)


def build_nc():
    nc = bacc.Bacc("TRN2", target_bir_lowering=False)

    xT_d = nc.dram_tensor("xT", [NX, S], F32, kind="ExternalInput")
    wqkv_d = nc.dram_tensor("Wqkv", [NX, 3 * NX], F32, kind="ExternalInput")
    bqkv_d = nc.dram_tensor("bqkv", [1, 3 * NX], F32, kind="ExternalInput")
    wproj_d = nc.dram_tensor("Wproj", [NX, NX], F32, kind="ExternalInput")
    bproj_d = nc.dram_tensor("bproj", [1, NX], F32, kind="ExternalInput")
    mask_d = nc.dram_tensor("cmask4", [4, P, 512], F32, kind="ExternalInput")
    ones_d = nc.dram_tensor("ones", [1, S], F32, kind="ExternalInput")
    vones_d = nc.dram_tensor("vones", [P, H], F32, kind="ExternalInput")
    sel8_d = nc.dram_tensor("sel8", [8, H, P], F32, kind="ExternalInput")
    outa_d = nc.dram_tensor("out_a", [S, NX], F32, kind="ExternalOutput")
    outk_d = nc.dram_tensor("out_kT", [NX, S], F32, kind="ExternalOutput")
    outv_d = nc.dram_tensor("out_v", [S, NX], F32, kind="ExternalOutput")

    with tile.TileContext(nc) as tc:
        with (
            tc.tile_pool(name="const", bufs=1) as constp,
            tc.tile_pool(name="qk", bufs=1, side="right") as qkp,
            tc.tile_pool(name="vpp", bufs=1, side="right") as vpp,
            tc.tile_pool(name="atp", bufs=1) as atp,
        ):
            # Memset cannot produce fp32r-rounded data, so all-ones constants
            # consumed by matmuls are DMA'd from DRAM inputs instead.
            ones_sb = constp.tile([1, S], F32, name="ones_sb")
            nc.sync.dma_start(out=_fr(ones_sb), in_=_fr(ones_d[:, :]))
            # full-width additive causal masks, one per diagonal position:
            # cols < q*128 get -1e4 (fully masked), the 128-wide diagonal
            # block gets the triangular mask, cols beyond it get 0
            # per-pair normalize selectors: sel8[c8][r, p] = 1 iff
            # r == 2*c8 + (p >= 64), so sel8[c8].T @ rall broadcasts the
            # pair's two recip rows onto its 128 partitions
            sel8_sb = [
                constp.tile([H, P], F32, name=f"sel8_{c}", tag=f"sel8_{c}")
                for c in range(8)
            ]
            for c in range(8):
                nc.sync.dma_start(out=_fr(sel8_sb[c]), in_=_fr(sel8_d[c]))
            # only the first (q+1)*128 columns of position-q's mask are
            # nonzero-interesting; columns right of the diagonal stay unmasked
            mask_sb = [
                constp.tile(
                    [P, (q + 1) * P], F32, name=f"mask_sb{q}", tag=f"mask_sb{q}"
                )
                for q in range(4)
            ]
            for q in range(4):
                nc.sync.dma_start(out=mask_sb[q], in_=mask_d[q, :, 0 : (q + 1) * P])

            qT = [qkp.tile([P, S], F32, name=f"qT{i}", tag=f"qT{i}") for i in range(8)]
            kT = [qkp.tile([P, S], F32, name=f"kT{i}", tag=f"kT{i}") for i in range(8)]
            vpad = [
                vpp.tile([P, H * E], F32, name=f"vp{i}", tag=f"vp{i}") for i in range(8)
            ]
            # head pair c: head 2c on partitions 0..63, head 2c+1 on 64..127
            aT = [atp.tile([P, S], F32, name=f"aT{c}", tag=f"aT{c}") for c in range(8)]

            # ---------------- stage A: qkv = x @ Wqkv + b ----------------
            with (
                tc.tile_pool(name="xp", bufs=1) as xp,
                tc.tile_pool(name="wgp", bufs=10) as wgp,
                tc.tile_pool(name="psA", bufs=5, space="PSUM") as psA,
            ):
                xT = []
                for n in range(8):
                    t = xp.tile([P, S], F32, name=f"xT{n}", tag=f"xT{n}")
                    nc.sync.dma_start(out=_fr(t), in_=_fr(xT_d[n * P : (n + 1) * P, :]))
                    xT.append(t)
                for t in range(8):
                    v3 = vpad[t].rearrange("p (h e) -> p h e", e=E)
                    nc.sync.dma_start(
                        out=_fr(v3[:, :, D : D + 1]), in_=_fr(vones_d[:, :, None])
                    )

                for mg in range(6):
                    wg = []
                    for n in range(8):
                        w = wgp.tile([P, 512], F32, name=f"wg_{mg}_{n}", tag="wg")
                        nc.sync.dma_start(
                            out=_fr(w),
                            in_=_fr(
                                wqkv_d[n * P : (n + 1) * P, mg * 512 : (mg + 1) * 512]
                            ),
                        )
                        wg.append(w)
                    bqg = wgp.tile([1, 512], F32, name=f"bqg_{mg}", tag="bqg", bufs=3)
                    nc.sync.dma_start(
                        out=_fr(bqg), in_=_fr(bqkv_d[:, mg * 512 : (mg + 1) * 512])
                    )
                    if mg < 4:
                        # q^T / k^T, transposed layout [m, s]
                        for ml in range(4):
                            m_abs = mg * 512 + ml * P
                            for j in range(2):
                                ps = psA.tile(
                                    [P, 512], F32, tag="pa", name=f"pa_{mg}_{ml}_{j}"
                                )
                                for n in range(8):
                                    nc.tensor.matmul(
                                        ps,
                                        _fr(wg[n][:, ml * P : (ml + 1) * P]),
                                        _fr(xT[n][:, j * 512 : (j + 1) * 512]),
                                        start=(n == 0),
                                        stop=False,
                                    )
                                nc.tensor.matmul(
                                    ps,
                                    _fr(bqg[:, ml * P : (ml + 1) * P]),
                                    _fr(ones_sb[:, j * 512 : (j + 1) * 512]),
                                    start=False,
                                    stop=True,
                                )
                                if m_abs < NX:
                                    # (1/sqrt(D) is applied later, inside the
                                    # exp activation's scale argument)
                                    nc.scalar.activation(
                                        _fr(qT[m_abs // P][:, j * 512 : (j + 1) * 512]),
                                        ps,
                                        AF.Copy,
                                    )
                                else:
                                    ki = (m_abs - NX) // P
                                    nc.vector.tensor_copy(
                                        _fr(kT[ki][:, j * 512 : (j + 1) * 512]), ps
                                    )
                            if m_abs >= NX:
                                ki = (m_abs - NX) // P
                                nc.sync.dma_start(
                                    out=outk_d[ki * P : (ki + 1) * P, :], in_=kT[ki]
                                )
                    else:
                        # v, natural layout [t, hd] with ones column per head
                        mvg = mg - 4
                        h0 = mvg * 8
                        for t in range(8):
                            ps = psA.tile([P, 512], F32, tag="pa", name=f"pv_{mg}_{t}")
                            for n in range(8):
                                nc.tensor.matmul(
                                    ps,
                                    _fr(xT[n][:, t * P : (t + 1) * P]),
                                    _fr(wg[n]),
                                    start=(n == 0),
                                    stop=False,
                                )
                            nc.tensor.matmul(
                                ps,
                                _fr(ones_sb[:, t * P : (t + 1) * P]),
                                _fr(bqg),
                                start=False,
                                stop=True,
                            )
                            v3 = vpad[t].rearrange("p (h e) -> p h e", e=E)
                            nc.vector.tensor_copy(
                                _fr(v3[:, h0 : h0 + 8, 0:D]),
                                ps.rearrange("p (h d) -> p h d", d=D),
                            )
                            if mvg == 1:
                                nc.sync.dma_start(
                                    out=outv_d[t * P : (t + 1) * P, :].rearrange(
                                        "p (h d) -> p h d", d=D
                                    ),
                                    in_=v3[:, :, 0:D],
                                )

            # ---------------- attention ----------------
            with (
                tc.tile_pool(name="rcp", bufs=4) as rcp,
                tc.tile_pool(name="etp", bufs=12) as etp,
                tc.tile_pool(name="psT", bufs=2, space="PSUM") as psT,
            ):
                # all 32 softmax denominators collect here (row = head), so a
                # single batched reciprocal replaces 32 serial one-lane ones
                den_all = atp.tile([H, S], F32, name="den_all")
                rall = atp.tile([H, S], F32, name="rall")
                # Head PAIRS are processed together: the even head's score
                # matmuls use PE rows 0..63 and the odd head's rows 64..127
                # (disjoint row groups), so adjacent even/odd matmuls run
                # CONCURRENTLY on the PE and keep the whole array active
                # (half-array activity lets the HAM clock-gate throttle).
                for c8 in range(8):
                    for j in range(2):
                        ntc = 4 * j + 4  # causal: t-chunks 0..4j+3
                        es = {0: [], 1: []}
                        offs = []
                        for c in range(ntc):
                            # columns left of the diagonal are fully masked;
                            # compute only the causal-valid range [lo, 512)
                            off = c * P - j * 512
                            lo = max(0, off)
                            offs.append(lo)
                            pss = {}
                            with tc.tile_critical():
                                for r in range(2):
                                    qr = r * D
                                    pss[r] = psT.tile(
                                        [P, 512],
                                        F32,
                                        tag=f"ps_s{r}",
                                        bufs=2,
                                        name=f"pss_{c8}_{j}_{c}_{r}",
                                    )
                                    nc.tensor.matmul(
                                        pss[r][:, lo:512],
                                        _fr(kT[c8][qr : qr + D, c * P : (c + 1) * P]),
                                        _fr(
                                            qT[c8][
                                                qr : qr + D,
                                                j * 512 + lo : (j + 1) * 512,
                                            ]
                                        ),
                                        start=True,
                                        stop=True,
                                    )
                            for r in range(2):
                                if off >= 0:  # diagonal block: tri mask add
                                    nc.vector.tensor_tensor(
                                        pss[r][:, off : off + P],
                                        pss[r][:, off : off + P],
                                        mask_sb[0],
                                        OP.add,
                                    )
                                e = etp.tile(
                                    [P, 512], F32, tag="et", name=f"e_{c8}_{j}_{c}_{r}"
                                )
                                # scale=0.125 applies the 1/sqrt(D) factor here
                                nc.scalar.activation(
                                    _fr(e[:, lo:512]),
                                    pss[r][:, lo:512],
                                    AF.Exp,
                                    scale=0.125,
                                )
                                es[r].append(e)
                        # context rows 0..63 + denominator row 64, via the
                        # [v|1] fused stationary (matmul dst at partition 0)
                        psc = {}
                        for r in range(2):
                            h = 2 * c8 + r
                            psc[r] = psT.tile(
                                [P, 512], F32, tag=f"ps_c{r}", bufs=2,
                                name=f"psc_{c8}_{j}_{r}",
                            )
                            for c in range(ntc):
                                lo = offs[c]
                                nc.tensor.matmul(
                                    psc[r][0:E, lo:512],
                                    _fr(vpad[c][:, h * E : h * E + E]),
                                    _fr(es[r][c][:, lo:512]),
                                    start=(c == 0),
                                    stop=(c == ntc - 1),
                                )
                        for r in range(2):
                            h = 2 * c8 + r
                            # stash the denominator row (ACT: PSUM row 64 ->
                            # SBUF row 64, then a tiny DMA to partition h of
                            # den_all) — keeps reciprocal off the PE path
                            rec = rcp.tile([P, 512], F32, tag="rec", name=f"rec_{h}_{j}")
                            nc.scalar.activation(
                                rec[D : D + 1, :], psc[r][D : D + 1, :], AF.Copy
                            )
                            nc.sync.dma_start(
                                out=den_all[h : h + 1, j * 512 : (j + 1) * 512],
                                in_=rec[D : D + 1, :],
                            )
                            # evict the unnormalized context rows
                            if r == 0:
                                nc.vector.tensor_copy(
                                    _fr(aT[c8][0:D, j * 512 : (j + 1) * 512]),
                                    psc[r][0:D, :],
                                )
                            else:
                                # odd head rows live on partitions 64..127 of
                                # the pair tile; DVE is lane-bound: stage + DMA
                                tmp = rcp.tile(
                                    [D, 512], F32, tag="tmp", name=f"tmp_{h}_{j}"
                                )
                                nc.vector.tensor_copy(_fr(tmp), psc[r][0:D, :])
                                nc.sync.dma_start(
                                    out=_fr(aT[c8][D : D + D, j * 512 : (j + 1) * 512]),
                                    in_=_fr(tmp),
                                )
                # one batched reciprocal over all (head, s) denominators
                with nc.allow_low_precision(
                    reason="fp32r rounding of softmax reciprocal rows"
                ):
                    nc.vector.reciprocal(_fr(rall), den_all)

            # normalize: K=16 selector matmul broadcasts each pair's two
            # recip rows across its 128 partitions, then one DVE multiply
            with tc.tile_pool(name="psN", bufs=2, space="PSUM") as psN:
                for c8 in range(8):
                    for j in range(2):
                        psb = psN.tile(
                            [P, 512], F32, tag="ps_b", bufs=2, name=f"psb_{c8}_{j}"
                        )
                        nc.tensor.matmul(
                            psb,
                            _fr(sel8_sb[c8]),
                            _fr(rall[:, j * 512 : (j + 1) * 512]),
                            start=True,
                            stop=True,
                        )
                        nc.vector.tensor_tensor(
                            _fr(aT[c8][:, j * 512 : (j + 1) * 512]),
                            aT[c8][:, j * 512 : (j + 1) * 512],
                            psb,
                            OP.mult,
                        )

            # ---------------- output projection ----------------
            with (
                tc.tile_pool(name="wpp", bufs=10) as wpp,
                tc.tile_pool(name="evp", bufs=4) as evp,
                tc.tile_pool(name="psP", bufs=4, space="PSUM") as psP,
            ):
                for mg in range(2):
                    wp = []
                    for n in range(8):
                        w = wpp.tile([P, 512], F32, name=f"wp_{mg}_{n}", tag="wp")
                        nc.sync.dma_start(
                            out=_fr(w),
                            in_=_fr(
                                wproj_d[n * P : (n + 1) * P, mg * 512 : (mg + 1) * 512]
                            ),
                        )
                        wp.append(w)
                    bpg = wpp.tile([1, 512], F32, name=f"bpg_{mg}", tag="bpg", bufs=2)
                    nc.sync.dma_start(
                        out=_fr(bpg), in_=_fr(bproj_d[:, mg * 512 : (mg + 1) * 512])
                    )
                    for i in range(8):
                        ps = psP.tile([P, 512], F32, tag="pp", name=f"pp_{mg}_{i}")
                        for n in range(8):
                            nc.tensor.matmul(
                                ps,
                                _fr(aT[n][:, i * P : (i + 1) * P]),
                                _fr(wp[n]),
                                start=(n == 0),
                                stop=False,
                            )
                        nc.tensor.matmul(
                            ps,
                            _fr(ones_sb[:, i * P : (i + 1) * P]),
                            _fr(bpg),
                            start=False,
                            stop=True,
                        )
                        ev = evp.tile([P, 512], F32, tag="ev", name=f"ev_{mg}_{i}")
                        nc.vector.tensor_copy(ev, ps)
                        nc.sync.dma_start(
                            out=outa_d[i * P : (i + 1) * P, mg * 512 : (mg + 1) * 512],
                            in_=ev,
                        )

    nc.compile()
    return nc


_NC_CACHE = None


def get_nc():
    global _NC_CACHE
    if _NC_CACHE is None:
        _NC_CACHE = build_nc()
    return _NC_CACHE


def make_in_maps(x, Wqkv, bqkv, Wproj, bproj):
    # cmask4[q][t, s]: additive mask for the diagonal-position-q score block
    # (psum block covers s columns q*128 aligned so that the diagonal sits at
    # columns [q*128, q*128+128))
    cols = np.arange(512)[None, :]
    rows = np.arange(P)[:, None]
    cmask4 = np.stack(
        [
            np.where(cols - q * P >= rows, 0.0, -10000.0).astype(np.float32)
            for q in range(4)
        ]
    )
    shared = {
        "Wqkv": np.ascontiguousarray(Wqkv, dtype=np.float32),
        "bqkv": np.ascontiguousarray(bqkv, dtype=np.float32).reshape(1, -1),
        "Wproj": np.ascontiguousarray(Wproj, dtype=np.float32),
        "bproj": np.ascontiguousarray(bproj, dtype=np.float32).reshape(1, -1),
        "cmask4": cmask4,
        "ones": np.ones((1, S), dtype=np.float32),
        "vones": np.ones((P, H), dtype=np.float32),
        "sel8": np.stack(
            [
                np.stack(
                    [
                        (np.full(P, r) == 2 * c + (np.arange(P) >= D)).astype(
                            np.float32
                        )
                        for r in range(H)
                    ]
                )
                for c in range(8)
            ]
        ),
    }
    return [
        {"xT": np.ascontiguousarray(np.asarray(x[b], dtype=np.float32).T), **shared}
        for b in range(B)
    ]


def assemble_outputs(results):
    a = np.stack([r["out_a"] for r in results])  # [B, S, NX]
    k = np.stack(
        [r["out_kT"].reshape(H, D, S).transpose(0, 2, 1) for r in results]
    )  # [B, H, S, D]
    v = np.stack(
        [r["out_v"].reshape(S, H, D).transpose(1, 0, 2) for r in results]
    )  # [B, H, S, D]
    present = np.stack([k, v])  # [2, B, H, S, D]
    return a, present


def kernel(x, Wqkv, bqkv, Wproj, bproj):
    from concourse.bass_utils import run_bass_kernel_spmd

    nc = get_nc()
    in_maps = make_in_maps(x, Wqkv, bqkv, Wproj, bproj)
    res = run_bass_kernel_spmd(nc, in_maps, core_ids=list(range(B)))
    return assemble_outputs(res.results)
